# revision 64
# baseline (speedup 1.0000x reference)
"""DiscreteMMSE Trainium2 Bass kernel.

Math (per batch row b):
  Z = data[b] @ W                      [N, T]   (W = squeeze(task_pool).T)
  resid = Z - targets[b][:, None]      [N, T]
  S'[i] = sum_{n<i} resid[n]^2         (strict cumsum over N; S'[0] = 0)
  E = exp(-0.5*S' - max_t(-0.5*S'))    (softmax-stable weights)
  out[b, i] = targets[b, i] + (sum_t E[i]*resid[i]) / (sum_t E[i])

Identical to the reference softmax-posterior MMSE prediction: the Gaussian
log-pdf constant and common shifts cancel in the softmax, and
pred = sum_t post*Z[i] = targets[i] - sum_t post*(targets[i]-Z[i]) collapses
onto resid. Row 0 (uniform prior over tasks) falls out of the strict cumsum.

Layout per NeuronCore (pure data parallel over B: 8 rows each, no collectives):
  - N=256 rows on partitions as two 128-row chunks; T=4096 on the free dim.
  - single-plane f32r matmuls (f32r streams 1 col/cycle vs fp32's 4); the
    ~2^-12 operand rounding this admits lands well inside the 2e-2 tolerance.
  - strict cumsum over N via triangular-ones f32r matmuls on TensorE:
    chunk0: U.T@sq0 ; chunk1: U.T@sq1 + ones.T@sq0, one PSUM group.
  - Four element-passes per [128,1024] tile (GPSIMD cannot touch PSUM, so
    they ride Act and DVE, balanced per the cost model):
      SQ    resid->resid^2 (f32r out)        Act Square
      EVAC  psum -> -0.5*S' + rowmax accum   DVE tensor_scalar (fused), or
            Act copy + all-SBUF DVE max (2x mode) for 2 of 8 tiles per b
      EXP   one full-row exp per (b,c); denominator lands via accum_out  Act
      NUM   E*resid + row-sum in one custom-DVE TENSOR_TENSOR_REDUCE     DVE
  - stage-2 resid is recomputed on TensorE (2 matmuls/tile) - cheaper than
    an extra elementwise store pass on the saturated vector engines.
  - E is stored bf16 (values in [0,1]; the fused reduce accumulates fp32),
    halving its SBUF footprint so three generations stay in flight.
  - PSUM (8 banks): tag "ps" = 3 slots of [128,1024] shared by resid rp,
    the stage-2 recompute rp2, and setup transposes - all freed by fast
    Act/DVE reads so DVE's evac never sits on the resid critical cycle;
    the cumsum sp has 1 slot (its reuse wait hides behind the interleaved
    fused-reduce tiles).
  - emission is a software pipeline over windows w (one per b), 4 rounds
    each: round r runs s1(w, r) with the fused-reduce tiles of (w-2, r)
    interleaved BETWEEN the two chunk evacs (so DVE always has ready work
    while Act runs the big exps of b=w-1, emitted at r=0 and r=3); the
    last two batches' reduce tiles pack into one drain window. Input DMAs
    are split per first-use, and the task-pool/data transposes stream into
    window 0's rounds (paired, evacuated off PSUM by Act, plus DVE at the
    round tails) so setup overlaps the pipeline fill.
"""

import numpy as np

B, N, D, T = 64, 256, 64, 4096
NCORES = 8
BPC = B // NCORES  # batch rows per core
NCH = 2            # partition chunks of N
PB = 128           # partitions per chunk
PT = 1024          # psum tile free size (2 banks)
MT = 512           # matmul moving free size (1 bank)
NJT = T // PT      # psum tiles per chunk row
NMM = PT // MT     # matmuls per psum tile

NTILE = BPC * NJT * NCH  # 64 elementwise tiles per core


def _evac_on_act(jt, c):
    # 2 of 8 tiles per b evacuate via Act + a cheap all-SBUF DVE max,
    # balancing Act vs DVE busy time (cost-model LP)
    return (jt, c) in ((0, 0), (2, 1))


_cached_nc = None


def _build():
    import concourse.bacc as bacc
    import concourse.mybir as mybir
    import concourse.tile as tile
    from concourse import masks
    from concourse.dve_ops import TENSOR_TENSOR_REDUCE as TTR_OP

    F32 = mybir.dt.float32
    BF16 = mybir.dt.bfloat16
    F32R = mybir.dt.float32r
    AF = mybir.ActivationFunctionType
    OP = mybir.AluOpType

    nc = bacc.Bacc("TRN2", debug=False)
    data_d = nc.dram_tensor("data", (BPC, N, D), F32, kind="ExternalInput")
    targ_d = nc.dram_tensor("targets", (BPC, N), F32, kind="ExternalInput")
    pool_d = nc.dram_tensor("task_pool", (T, D), F32, kind="ExternalInput")
    out_d = nc.dram_tensor("out", (BPC, N), F32, kind="ExternalOutput")

    NW = T // PB  # 32 task-pool transpose chunks

    with tile.TileContext(nc) as tc:
        with (
            tc.tile_pool(name="const", bufs=1) as const,
            tc.tile_pool(name="ld", bufs=1) as ld,
            tc.tile_pool(name="sq32p", bufs=4) as sq32p,
            tc.tile_pool(name="avp", bufs=2) as avp,
            tc.tile_pool(name="ep", bufs=3) as ep,
            tc.tile_pool(name="psp", bufs=2) as psp,
            tc.tile_pool(name="small", bufs=4) as small,
            # one PSUM pool (8 banks): tag "ps" = 3 slots of [128,1024]
            # shared by resid rp0/rp1, the stage-2 recompute rp2, and the
            # setup transposes (all freed by fast Act/DVE reads, keeping
            # DVE's evac OFF the resid critical cycle); tag "sp" = 1 slot
            # for the cumsum (its reuse wait hides behind interleaved TTRs)
            tc.tile_pool(name="psum", bufs=3, space="PSUM") as pspool,
        ):
            utri = const.tile([PB, PB], F32R)     # strictly-upper ones (lhsT)
            onesm = const.tile([PB, PB], F32R)    # all-ones
            waug = const.tile([D + 1, T], F32R)        # [W ; -1]
            daug = const.tile([D + 1, BPC * N], F32R)  # [data.T ; targets]
            tpart = [const.tile([PB, BPC], F32, name=f"tpart{c}", tag=f"tpart{c}") for c in range(NCH)]
            den = [const.tile([PB, BPC], F32, name=f"den{c}", tag=f"den{c}") for c in range(NCH)]
            num = [const.tile([PB, BPC], F32, name=f"num{c}", tag=f"num{c}") for c in range(NCH)]

            # ---- input DMAs (few, large) + constants ----
            trow = ld.tile([1, BPC * N], F32, tag="trow", name="trow")
            KQ = NW // 4
    
            wbigs = [
                ld.tile([PB, KQ * D], F32, tag=f"wbig{q}", name="wbig")
                for q in range(4)
            ]
            dbig0 = ld.tile([PB, NCH * D], F32, tag="dbig0", name="dbig0")
            dbigr = ld.tile(
                [PB, (BPC - 1) * NCH * D], F32, tag="dbigr", name="dbigr"
            )
            nc.sync.dma_start(
                dbig0[:].rearrange("p (c d) -> p c d", d=D),
                data_d[0].rearrange("(c p) d -> p c d", p=PB),
            )
            nc.sync.dma_start(
                wbigs[0][:].rearrange("p (k d) -> p k d", d=D),
                pool_d[0 : KQ * PB].rearrange("(k p) d -> p k d", p=PB),
            )
            nc.sync.dma_start(trow[:, 0:N], targ_d[0:1, :])
            for q in range(1, 4):
                nc.sync.dma_start(
                    wbigs[q][:].rearrange("p (k d) -> p k d", d=D),
                    pool_d[q * KQ * PB : (q + 1) * KQ * PB].rearrange(
                        "(k p) d -> p k d", p=PB
                    ),
                )
            nc.sync.dma_start(
                dbigr[:].rearrange("p (b c d) -> p b c d", d=D, c=NCH),
                data_d[1:].rearrange("b (c p) d -> p b c d", p=PB),
            )
            for b in range(1, BPC):
                nc.sync.dma_start(
                    trow[:, b * N : (b + 1) * N], targ_d[b : b + 1, :]
                )
            for c in range(NCH):
                tv = targ_d[:, c * PB : (c + 1) * PB].rearrange("b p -> p b")
                nc.sync.dma_start(tpart[c][:], tv)
            ident = ld.tile([PB, PB], F32, tag="ident", name="ident")
            masks.make_identity(nc, ident[:])
            utri_f = ld.tile([PB, PB], F32, tag="utri_f", name="utri_f")
            masks.make_upper_triangular(nc, utri_f[:], 1.0, diag=False)
            nc.any.memset(onesm[:].bitcast(F32), 1.0)
            nc.gpsimd.tensor_copy(utri[:], utri_f[:])
            for b in range(BPC):
                nc.gpsimd.tensor_copy(
                    daug[D : D + 1, b * N : (b + 1) * N],
                    trow[:, b * N : (b + 1) * N],
                )
            nc.any.memset(waug[D : D + 1, :].bitcast(F32), -1.0)

            def emit_wchunk2(k):
                """transpose task-pool chunks k,k+1 into waug; the pair
                shares one PSUM tile so Act evacuates both in one copy."""
                pt = pspool.tile([D, 2 * PB], F32, tag="ps", name="pt")
                for j in range(2):
                    src_t = wbigs[(k + j) // KQ]
                    o = ((k + j) % KQ) * D
                    nc.tensor.transpose(
                        pt[:, j * PB : (j + 1) * PB], src_t[:, o : o + D], ident[:]
                    )
                if k >= 10:
                    nc.vector.tensor_copy(waug[0:D, k * PB : (k + 2) * PB], pt[:])
                else:
                    nc.scalar.copy(waug[0:D, k * PB : (k + 2) * PB], pt[:])

            def emit_dchunk(b):
                """transpose data row b (both chunks) into daug, one copy."""
                pt = pspool.tile([D, 2 * PB], F32, tag="ps", name="pt")
                for c in range(NCH):
                    if b == 0:
                        src_t, o = dbig0, c * D
                    else:
                        src_t, o = dbigr, ((b - 1) * NCH + c) * D
                    nc.tensor.transpose(
                        pt[:, c * PB : (c + 1) * PB], src_t[:, o : o + D], ident[:]
                    )
                nc.scalar.copy(daug[0:D, b * N : (b + 1) * N], pt[:])

            # setup chunks streamed into early windows: (w, r) -> emitters
            SETUP = {}
            SETUP.setdefault((0, 0), []).extend(
                [(emit_wchunk2, 4), (emit_wchunk2, 6), (emit_wchunk2, 8)]
            )
            SETUP_END = {}
            for k in range(10, NW, 2):
                r = min(2, (k - 10) // 8)  # pairs at end of w0 r0-r2 (DVE)
                SETUP_END.setdefault((0, r), []).append((emit_wchunk2, k))
            SETUP.setdefault((0, 2), []).append((emit_dchunk, 1))
            SETUP.setdefault((0, 3), []).append((emit_dchunk, 2))
            for b in range(3, BPC):
                SETUP.setdefault((b - 2, 3), []).append((emit_dchunk, b))

            # ---- per-window state ----
            av_s, mx2_s, nbs_s, evs_s, num4_s = {}, {}, {}, {}, {}
            den2_s = {}

            def s1_resid_sq(b, jt):
                av, mx2 = av_s[b], mx2_s[b]
                sqs = []
                for c in range(NCH):
                    cs = slice(b * N + c * PB, b * N + (c + 1) * PB)
                    rp = pspool.tile([PB, PT], F32, tag="ps", name="rp")
                    for h in range(NMM):
                        lo_ = jt * PT + h * MT
                        nc.tensor.matmul(
                            rp[:, h * MT : (h + 1) * MT],
                            daug[:, cs],
                            waug[:, lo_ : lo_ + MT],
                            start=True, stop=True,
                        )
                    sq = sq32p.tile([PB, PT], F32R, tag=f"sq{c}", name=f"sq{c}")
                    nc.scalar.activation(sq[:], rp[:], AF.Square)
                    sqs.append(sq)
                return sqs

            def s1_cumsum_evac(b, jt, c, sqs):
                av, mx2 = av_s[b], mx2_s[b]
                js = slice(jt * PT, (jt + 1) * PT)
                sp = pspool.tile([PB, PT], F32, tag="sp", bufs=1, name="sp")
                for h in range(NMM):
                    hsl = slice(h * MT, (h + 1) * MT)
                    nc.tensor.matmul(
                        sp[:, hsl], utri[:], sqs[c][:, hsl],
                        start=True, stop=(c == 0),
                    )
                    if c == 1:
                        nc.tensor.matmul(
                            sp[:, hsl], onesm[:], sqs[0][:, hsl],
                            start=False, stop=True,
                        )
                if _evac_on_act(jt, c):
                    # Act evacuates; cheap all-SBUF DVE pass (2x mode)
                    # recovers the tile max into mx2.
                    nc.scalar.mul(av[c][:, js], sp[:], -0.5)
                    pscr = psp.tile([PB, PT], F32, tag="pscr", name="pscr")
                    nc.vector.tensor_scalar(
                        out=pscr[:],
                        in0=av[c][:, js],
                        scalar1=1.0,
                        scalar2=None,
                        op0=OP.mult,
                        op1=OP.max,
                        accum_out=mx2[c][:, jt : jt + 1],
                    )
                else:
                    nc.vector.tensor_scalar(
                        out=av[c][:, js],
                        in0=sp[:],
                        scalar1=-0.5,
                        scalar2=None,
                        op0=OP.mult,
                        op1=OP.max,
                        accum_out=mx2[c][:, jt : jt + 1],
                    )

            def s1_finish(b):
                """negated row-max once all evac partials of b landed."""
                nbs = []
                for c in range(NCH):
                    nb = small.tile([PB, 1], F32, tag=f"nb{c}", name=f"nb{c}")
                    nc.vector.tensor_reduce(
                        nb[:], mx2_s[b][c][:], axis=mybir.AxisListType.X,
                        op=OP.max, negate=True,
                    )
                    nbs.append(nb)
                nbs_s[b] = nbs

            def s2_exp(b, c):
                """exp over av (two 2048 halves so Act's stream stays fine-
                grained); denominator partials land via accum_out."""
                ev = ep.tile([PB, T], BF16, tag=f"E{c}", name=f"E{c}")
                nc.scalar.activation(
                    ev[:],
                    av_s[b][c][:],
                    AF.Exp,
                    bias=nbs_s[b][c][:],
                    scale=1.0,
                    accum_out=den[c][:, b : b + 1],
                )
                evs_s[b][c] = ev

            def s2_ttr(b, jt, c):
                """resid recompute + fused E*resid row-sum for one chunk."""
                js = slice(jt * PT, (jt + 1) * PT)
                cs = slice(b * N + c * PB, b * N + (c + 1) * PB)
                rp2 = pspool.tile([PB, PT], F32, tag="ps", name="rp2")
                for h in range(NMM):
                    lo_ = jt * PT + h * MT
                    nc.tensor.matmul(
                        rp2[:, h * MT : (h + 1) * MT],
                        daug[:, cs],
                        waug[:, lo_ : lo_ + MT],
                    )
                ev = evs_s[b][c]
                nc.vector._custom_dve(
                    TTR_OP,
                    out=ev[:, js],
                    in0=ev[:, js],
                    in1=rp2[:],
                    s0=0.0,
                    s1=1.0,
                    accum_out=num4_s[b][c][:, jt : jt + 1],
                )

            def s2_finish(b):
                for c in range(NCH):
                    nc.vector.tensor_reduce(
                        num[c][:, b : b + 1], num4_s[b][c][:],
                        axis=mybir.AxisListType.X, op=OP.add,
                    )


            # ---- pre-main setup: just enough for window 0 round 0 ----
            emit_dchunk(0)
            emit_wchunk2(0)
            emit_wchunk2(2)

            # ---- depth-2 software-pipelined windows ----
            for w in range(BPC + 1):
                if w < BPC:
                    av_s[w] = [
                        avp.tile([PB, T], F32, tag=f"av{c}", name=f"av{c}")
                        for c in range(NCH)
                    ]
                    mx2_s[w] = [
                        small.tile([PB, NJT], F32, tag=f"mx2{c}", name=f"mx2{c}")
                        for c in range(NCH)
                    ]
                for r in range(NJT):
                    for fn, arg in SETUP.get((w, r), []) or []:
                        fn(arg)
                    # fused-reduce schedule: batch b runs one window behind
                    # its exp; the last batch's tiles pack into the final
                    # window alongside batch BPC-2's to shorten the drain
                    if w < BPC:
                        ttrs = [(w - 2, r)] if w - 2 >= 0 else []
                    else:
                        ttrs = {
                            0: [(BPC - 2, 0)],
                            1: [(BPC - 2, 1), (BPC - 1, 0)],
                            2: [(BPC - 2, 2), (BPC - 1, 1)],
                            3: [(BPC - 2, 3), (BPC - 1, 2), (BPC - 1, 3)],
                        }[r]
                    sqs = None
                    if w < BPC:
                        sqs = s1_resid_sq(w, r)
                        s1_cumsum_evac(w, r, 0, sqs)
                    exp_rounds = (0, 3) if w < BPC else (0, 1)
                    if r in exp_rounds and 0 <= w - 1 < BPC:
                        s2_exp(w - 1, exp_rounds.index(r))
                    for tb, tj in ttrs:
                        s2_ttr(tb, tj, 0)
                    if sqs is not None:
                        s1_cumsum_evac(w, r, 1, sqs)
                    for tb, tj in ttrs:
                        s2_ttr(tb, tj, 1)
                        if tj == 3:
                            s2_finish(tb)
                    for fn, arg in SETUP_END.get((w, r), []) or []:
                        fn(arg)
                if w < BPC:
                    s1_finish(w)
                    evs_s[w] = [None, None]
                    num4_s[w] = [
                        small.tile([PB, NJT], F32, tag=f"num4{c}", name=f"num4{c}")
                        for c in range(NCH)
                    ]


            # ---- finals: out = targets + num/den ----
            for c in range(NCH):
                rec = small.tile([PB, BPC], F32, tag=f"rec{c}", name=f"rec{c}")
                prod = small.tile([PB, BPC], F32, tag=f"prod{c}", name=f"prod{c}")
                outv = small.tile([PB, BPC], F32, tag=f"outv{c}", name=f"outv{c}")
                nc.vector.reciprocal(rec[:], den[c][:])
                nc.vector.tensor_mul(prod[:], num[c][:], rec[:])
                nc.vector.tensor_add(outv[:], tpart[c][:], prod[:])
                ov = out_d[:, c * PB : (c + 1) * PB].rearrange("b p -> p b")
                nc.sync.dma_start(ov, outv[:])

    nc.compile()
    return nc


def _get_nc():
    global _cached_nc
    if _cached_nc is None:
        _cached_nc = _build()
    return _cached_nc


_cached_runner = None


def _get_runner():
    """Build once: a cached jax.jit shard_map over the 8 NeuronCores.

    run_bass_kernel_spmd/run_bass_via_pjrt construct a fresh jax.jit closure
    per call (full retrace); caching the callable keeps repeat calls cheap.
    """
    global _cached_runner
    if _cached_runner is None:
        import jax
        from jax.sharding import Mesh, PartitionSpec
        from concourse import bass2jax
        from concourse.bass2jax import _bass_exec_p, partition_id_tensor
        import concourse.mybir as mybir

        try:
            from jax.experimental.shard_map import shard_map
        except ImportError:
            from jax.shard_map import shard_map

        bass2jax.install_neuronx_cc_hook()
        nc = _get_nc()
        partition_name = (
            nc.partition_id_tensor.name if nc.partition_id_tensor else None
        )
        in_names, out_names, out_avals, zero_outs = [], [], [], []
        for alloc in nc.m.functions[0].allocations:
            if not isinstance(alloc, mybir.MemoryLocationSet):
                continue
            name = alloc.memorylocations[0].name
            if alloc.kind == "ExternalInput":
                if name != partition_name:
                    in_names.append(name)
            elif alloc.kind == "ExternalOutput":
                out_names.append(name)
                shape = tuple(alloc.tensor_shape)
                dtype = mybir.dt.np(alloc.dtype)
                out_avals.append(jax.core.ShapedArray(shape, dtype))
                zero_outs.append(np.zeros((NCORES * shape[0], *shape[1:]), dtype))
        n_params = len(in_names)
        all_names = list(in_names) + list(out_names)
        if partition_name is not None:
            all_names.append(partition_name)
        donate = tuple(range(n_params, n_params + len(out_names)))

        def _body(*args):
            operands = list(args)
            if partition_name is not None:
                operands.append(partition_id_tensor())
            return tuple(
                _bass_exec_p.bind(
                    *operands,
                    out_avals=tuple(out_avals),
                    in_names=tuple(all_names),
                    out_names=tuple(out_names),
                    lowering_input_output_aliases=(),
                    sim_require_finite=True,
                    sim_require_nnan=True,
                    nc=nc,
                )
            )

        devices = jax.devices()[:NCORES]
        mesh = Mesh(np.asarray(devices), ("core",))
        in_specs = tuple(
            PartitionSpec() if name == "task_pool" else PartitionSpec("core")
            for name in in_names
        ) + (PartitionSpec("core"),) * len(out_names)
        sharded = jax.jit(
            shard_map(
                _body,
                mesh=mesh,
                in_specs=in_specs,
                out_specs=(PartitionSpec("core"),) * len(out_names),
                check_rep=False,
            ),
            donate_argnums=donate,
            keep_unused=True,
        )
        _cached_runner = (sharded, in_names, out_names, out_avals, zero_outs)
    return _cached_runner


def _kernel_fallback(data, targets, tp):
    """Robust path via the stock SPMD runner (fresh jit each call)."""
    from concourse.bass_utils import run_bass_kernel_spmd

    nc = _get_nc()
    in_maps = [
        {
            "data": data[i * BPC : (i + 1) * BPC],
            "targets": targets[i * BPC : (i + 1) * BPC],
            "task_pool": tp,
        }
        for i in range(NCORES)
    ]
    res = run_bass_kernel_spmd(nc, in_maps, core_ids=list(range(NCORES)))
    return np.concatenate([r["out"] for r in res.results], axis=0)


def kernel(data, targets, task_pool, **_):
    data = np.ascontiguousarray(np.asarray(data, np.float32))
    targets = np.ascontiguousarray(np.asarray(targets, np.float32))
    tp = np.ascontiguousarray(np.asarray(task_pool, np.float32).reshape(T, D))

    try:
        sharded, in_names, out_names, out_avals, zero_outs = _get_runner()
        full = {
            "data": data.reshape(NCORES * BPC, N, D),
            "targets": targets.reshape(NCORES * BPC, N),
            "task_pool": tp,
        }
        args = [full[name] for name in in_names]
        args += [np.zeros_like(z) for z in zero_outs]
        outs = sharded(*args)
        out = np.asarray(outs[out_names.index("out")])
        return out.reshape(B, N)
    except Exception:
        return _kernel_fallback(data, targets, tp)
